# revision 17
# baseline (speedup 1.0000x reference)
import sys

sys.path.insert(0, "/opt/trn_rl_repo")

import numpy as np

from concourse import bacc, bass, mybir, tile
from concourse.bass_utils import run_bass_kernel_spmd

# Problem constants (hardcoded; see spec)
B, K, H, Ch = 2, 32, 64, 32
NT, NX = 64, 256
Q = NT * NX            # 16384 queries per batch element
NCORES = 8
QC = Q * B // NCORES   # 4096 queries per core (q-slab)
HID = 2 * H            # 128: score-hidden(64) | value-hidden(64)

F32 = mybir.dt.float32
F32R = mybir.dt.float32r
AF = mybir.ActivationFunctionType
ALU = mybir.AluOpType
AX = mybir.AxisListType


def _build_nc(tanh_bias):
    nc = bacc.Bacc(None, target_bir_lowering=False)

    xc = nc.dram_tensor("xc", [QC, K, Ch], F32R, kind="ExternalInput")
    w14 = nc.dram_tensor("w14", [128, HID], F32R, kind="ExternalInput")
    w2x = nc.dram_tensor("w2x", [HID, 32], F32R, kind="ExternalInput")
    segb = nc.dram_tensor("segb", [HID, K], F32, kind="ExternalInput")
    maskt = nc.dram_tensor("maskt", [128, K], F32, kind="ExternalInput")
    ident = nc.dram_tensor("ident", [128, 128], F32R, kind="ExternalInput")

    wout = nc.dram_tensor("wout", [QC, K], F32, kind="ExternalOutput")
    rout = nc.dram_tensor("rout", [QC], F32, kind="ExternalOutput")

    scr_s = nc.dram_tensor("scr_s", [K * QC], F32, kind="Internal")
    scr_v = nc.dram_tensor("scr_v", [K * QC], F32, kind="Internal")

    with tile.TileContext(nc) as tc:
        with (
            tc.tile_pool(name="const", bufs=1) as cpool,
            tc.tile_pool(name="tn", bufs=3) as tn_pool,
            tc.tile_pool(name="xt", bufs=3) as xt_pool,
            tc.tile_pool(name="g", bufs=3) as g_pool,
            tc.tile_pool(name="sv", bufs=2) as sv_pool,
            tc.tile_pool(name="dense", bufs=4) as d_pool,
            tc.tile_pool(name="dsmall", bufs=4) as s_pool,
            tc.tile_pool(name="pt", bufs=2, space="PSUM") as pt_pool,
            tc.tile_pool(name="py", bufs=1, space="PSUM") as py_pool,
            tc.tile_pool(name="ps", bufs=1, space="PSUM") as ps_pool,
        ):
            w14_sb = cpool.tile([128, HID], F32R)
            w2_sb = cpool.tile([HID, 32], F32R)
            segb_sb = cpool.tile([HID, K], F32)
            mask_sb = cpool.tile([128, K], F32)
            id_sb = cpool.tile([128, 128], F32R)
            nc.sync.dma_start(w14_sb[:], w14[:])
            nc.sync.dma_start(w2_sb[:], w2x[:])
            nc.sync.dma_start(segb_sb[:], segb[:])
            nc.sync.dma_start(mask_sb[:], maskt[:])
            nc.sync.dma_start(id_sb[:], ident[:])


            scv = scr_s[:].rearrange("(k h u n) -> k h u n", k=K, h=2, u=4)
            vcv = scr_v[:].rearrange("(k h u n) -> k h u n", k=K, h=2, u=4)

            sc_state = {"buf": None, "base": -1}

            def _emit_mm2(item):
                g_sb, k, hh = item
                sg = 2 * k + hh
                g_v = g_sb[:].rearrange("p (j m) -> p j m", j=4)
                psum_s = ps_pool.tile([8, 512], F32, tag="ps")
                for u in range(4):
                    nc.tensor.matmul(
                        psum_s[:],
                        w2_sb[:, 8 * u : 8 * u + 8],
                        g_v[:, :, 128 * u : 128 * u + 128],
                        start=(u == 0), stop=(u == 3),
                    )
                if sc_state["buf"] is None:
                    sc_state["buf"] = sv_pool.tile([8, 2048], F32, tag="sc", name="sc_big")
                    sc_state["base"] = sg
                sl = sg - sc_state["base"]
                nc.vector.tensor_copy(
                    sc_state["buf"][:, 512 * sl : 512 * sl + 512], psum_s[:]
                )
                if sl == 3:
                    buf = sc_state["buf"]
                    k0 = sc_state["base"] // 2
                    # dst dims (u:partition, k:2, h:2, n:512); src free (sgl, n)
                    d_s = scv[k0 : k0 + 2].rearrange("k h u n -> u k h n")
                    d_v = vcv[k0 : k0 + 2].rearrange("k h u n -> u k h n")
                    s_s = buf[0:4, :].rearrange("u (kr h n) -> u kr h n", kr=2, h=2)
                    s_v = buf[4:8, :].rearrange("u (kr h n) -> u kr h n", kr=2, h=2)
                    nc.sync.dma_start(d_s, s_s)
                    nc.sync.dma_start(d_v, s_v)
                    sc_state["buf"] = None

            prev = []
            # ---- Phase A: per k: both halves: transpose -> mm1 -> gelu -> mm2
            for k in range(K):
                tnat = tn_pool.tile([128, 1024], F32R)  # free = (h,t,j,c)
                src = xc[:, k : k + 1, :].rearrange(
                    "(h t j p) kk c -> p h t j kk c", h=2, t=4, j=4, p=128
                ).squeeze(4)
                dst = tnat[:].rearrange("p (h t j c) -> p h t j c", h=2, t=4, j=4)
                nc.sync.dma_start(dst, src)

                for hh in range(2):
                    sg = k * 2 + hh
                    tview = tnat[:, 512 * hh : 512 * hh + 512]

                    psum_t = pt_pool.tile([128, 512], F32)
                    for t in range(4):
                        nc.tensor.transpose(
                            psum_t[:, 128 * t : 128 * t + 128].bitcast(F32R),
                            tview[:, 128 * t : 128 * t + 128],
                            id_sb[:],
                        )
                    x_t = xt_pool.tile([128, 512], F32R)
                    nc.vector.tensor_copy(x_t[:], psum_t[:].bitcast(F32R))

                    psum_y = py_pool.tile([128, 2048], F32)
                    for j in range(4):
                        nc.tensor.matmul(
                            psum_y[:, 512 * j : 512 * j + 512],
                            w14_sb[32 * j : 32 * j + 32, :],
                            x_t[32 * j : 32 * j + 32, :],
                            tile_position=(32 * j, 0),
                        )

                    g_sb = g_pool.tile([128, 2048], F32R)
                    nc.scalar.activation(
                        g_sb[:], psum_y[:], AF.Gelu, bias=segb_sb[:, k : k + 1]
                    )

                    prev.append((g_sb, k, hh))
                    if len(prev) > 1:
                        _emit_mm2(prev.pop(0))

            while prev:
                _emit_mm2(prev.pop(0))

            # ---- Phase B: read back dense
            # dense tiles: memory layout f = 32*k + ql (k outer)
            ds = d_pool.tile([128, K * 32], F32, tag="dense")
            dv = d_pool.tile([128, K * 32], F32, tag="dense")
            nc.sync.dma_start(
                ds[:].rearrange("p (k ql) -> p k ql", k=K),
                scr_s[:].rearrange("(k p ql) -> p k ql", k=K, p=128),
            )
            nc.sync.dma_start(
                dv[:].rearrange("p (k ql) -> p k ql", k=K),
                scr_v[:].rearrange("(k p ql) -> p k ql", k=K, p=128),
            )

            p_exp = d_pool.tile([128, K * 32], F32, tag="dense")
            t_val = d_pool.tile([128, K * 32], F32, tag="dense")
            nc.scalar.activation(p_exp[:], ds[:], AF.Exp)
            nc.scalar.activation(t_val[:], dv[:], AF.Tanh, scale=0.5, bias=tanh_bias)

            w_un = d_pool.tile([128, K * 32], F32, tag="dense")
            nc.vector.tensor_tensor(
                w_un[:].rearrange("p (k ql) -> p k ql", k=K),
                p_exp[:].rearrange("p (k ql) -> p k ql", k=K),
                mask_sb[:].unsqueeze(2).broadcast_to([128, K, 32]),
                ALU.mult,
            )
            zsum = s_pool.tile([128, 32], F32, tag="small")
            nc.vector.tensor_reduce(
                zsum[:],
                w_un[:].rearrange("p (k ql) -> p ql k", k=K),
                AX.X, ALU.add,
            )
            zrec = s_pool.tile([128, 32], F32, tag="small")
            nc.vector.reciprocal(zrec[:], zsum[:])

            # wn memory layout f = 32*ql + k (matches wout row order)
            wn = d_pool.tile([128, K * 32], F32, tag="dense")
            nc.vector.tensor_tensor(
                wn[:].rearrange("p (ql k) -> p ql k", k=K),
                w_un[:].rearrange("p (k ql) -> p ql k", k=K),
                zrec[:].unsqueeze(2).broadcast_to([128, 32, K]),
                ALU.mult,
            )
            wt = d_pool.tile([128, K * 32], F32, tag="dense")
            nc.vector.tensor_tensor(
                wt[:].rearrange("p (ql k) -> p ql k", k=K),
                wn[:].rearrange("p (ql k) -> p ql k", k=K),
                t_val[:].rearrange("p (k ql) -> p ql k", k=K),
                ALU.mult,
            )
            ssum = s_pool.tile([128, 32], F32, tag="small")
            nc.vector.tensor_reduce(
                ssum[:], wt[:].rearrange("p (ql k) -> p ql k", k=K), AX.X, ALU.add
            )
            rho = s_pool.tile([128, 32], F32, tag="small")
            nc.vector.tensor_scalar(rho[:], ssum[:], 0.5, 0.5, ALU.mult, ALU.add)

            nc.sync.dma_start(
                wout[:].rearrange("(p ql) k -> p (ql k)", p=128), wn[:]
            )
            nc.sync.dma_start(rout[:].rearrange("(p ql) -> p ql", p=128), rho[:])

    nc.compile()
    return nc


_CACHE = {}
LAST_RESULT = None


def kernel(
    seg_emb, char_feat, pieces_mask, Ws1, bs1, Ws2, bs2,
    Wv1, bv1, Wv2, bv2, log_temperature, nt, nx,
):
    seg_emb = np.asarray(seg_emb, np.float32)
    char_feat = np.asarray(char_feat, np.float32)
    mask = np.asarray(pieces_mask)
    Ws1 = np.asarray(Ws1, np.float32); bs1 = np.asarray(bs1, np.float32)
    Ws2 = np.asarray(Ws2, np.float32); bs2 = np.asarray(bs2, np.float32)
    Wv1 = np.asarray(Wv1, np.float32); bv1 = np.asarray(bv1, np.float32)
    Wv2 = np.asarray(Wv2, np.float32); bv2 = np.asarray(bv2, np.float32)
    T = float(np.exp(np.float32(log_temperature)))

    maskf = mask.astype(np.float32)                      # (B, K)
    seg = seg_emb * maskf[:, :, None]                    # (B, K, H)
    W1comb = np.concatenate([Ws1[H:], Wv1[H:]], axis=1)  # (32, 128)
    w14 = np.ascontiguousarray(np.tile(W1comb, (4, 1)), dtype=np.float32)
    w2x = np.zeros((HID, 32), np.float32)
    for u in range(4):
        w2x[:H, 8 * u + u] = -Ws2[:, 0] / T
        w2x[H:, 8 * u + 4 + u] = Wv2[:, 0]
    ident = np.eye(128, dtype=np.float32)
    segb_all = []
    for b in range(B):
        sb_s = seg[b] @ Ws1[:H] + bs1                    # (K, 64)
        sb_v = seg[b] @ Wv1[:H] + bv1                    # (K, 64)
        segb_all.append(
            np.ascontiguousarray(np.concatenate([sb_s, sb_v], axis=1).T,
                                 dtype=np.float32))      # (128, K)

    tanh_bias = 0.5 * float(bv2[0])
    if tanh_bias not in _CACHE:
        _CACHE[tanh_bias] = _build_nc(tanh_bias)
    nc = _CACHE[tanh_bias]

    in_maps = []
    for c in range(NCORES):
        b = c // (NCORES // B)
        q0 = (c % (NCORES // B)) * QC
        in_maps.append({
            "xc": np.ascontiguousarray(char_feat[b, q0 : q0 + QC]),
            "w14": w14,
            "w2x": w2x,
            "segb": segb_all[b],
            "maskt": np.ascontiguousarray(
                np.tile(maskf[b][None, :], (128, 1)), dtype=np.float32),
            "ident": ident,
        })

    import os
    kw = {}
    if os.environ.get("BASS_TRACE"):
        kw = dict(trace=True, trace_cores=[0])
    global LAST_RESULT
    LAST_RESULT = run_bass_kernel_spmd(nc, in_maps, list(range(NCORES)), **kw)
    res = LAST_RESULT.results

    weights = np.empty((B, Q, K), np.float32)
    rho = np.empty((B, Q), np.float32)
    for c in range(NCORES):
        b = c // (NCORES // B)
        q0 = (c % (NCORES // B)) * QC
        weights[b, q0 : q0 + QC] = res[c]["wout"]
        rho[b, q0 : q0 + QC] = res[c]["rout"]

    output_grid = rho.reshape(B, 1, NT, NX)
    selection_weights = weights.reshape(B, NT, NX, K)
    return output_grid, selection_weights


# revision 18
# speedup vs baseline: 1.1711x; 1.1711x over previous
import sys

sys.path.insert(0, "/opt/trn_rl_repo")

import numpy as np

from concourse import bacc, bass, mybir, tile
from concourse.bass_utils import run_bass_kernel_spmd

# Problem constants (hardcoded; see spec)
B, K, H, Ch = 2, 32, 64, 32
NT, NX = 64, 256
Q = NT * NX            # 16384 queries per batch element
NCORES = 8
QC = Q * B // NCORES   # 4096 queries per core (q-slab)
HID = 2 * H            # 128: score-hidden(64) | value-hidden(64)

F32 = mybir.dt.float32
F32R = mybir.dt.float32r
AF = mybir.ActivationFunctionType
ALU = mybir.AluOpType
AX = mybir.AxisListType


def _build_nc(tanh_bias):
    nc = bacc.Bacc(None, target_bir_lowering=False)

    xc = nc.dram_tensor("xc", [QC, K, Ch], F32R, kind="ExternalInput")
    w14 = nc.dram_tensor("w14", [128, HID], F32R, kind="ExternalInput")
    w2x = nc.dram_tensor("w2x", [HID, 32], F32R, kind="ExternalInput")
    segb = nc.dram_tensor("segb", [HID, K], F32, kind="ExternalInput")
    maskt = nc.dram_tensor("maskt", [128, K], F32, kind="ExternalInput")
    ident = nc.dram_tensor("ident", [128, 128], F32R, kind="ExternalInput")

    wout = nc.dram_tensor("wout", [QC, K], F32, kind="ExternalOutput")
    rout = nc.dram_tensor("rout", [QC], F32, kind="ExternalOutput")

    scr_s = nc.dram_tensor("scr_s", [K * QC], F32, kind="Internal")
    scr_v = nc.dram_tensor("scr_v", [K * QC], F32, kind="Internal")

    with tile.TileContext(nc) as tc:
        with (
            tc.tile_pool(name="const", bufs=1) as cpool,
            tc.tile_pool(name="tn", bufs=3) as tn_pool,
            tc.tile_pool(name="xt", bufs=3) as xt_pool,
            tc.tile_pool(name="g", bufs=3) as g_pool,
            tc.tile_pool(name="sv", bufs=2) as sv_pool,
            tc.tile_pool(name="dense", bufs=4) as d_pool,
            tc.tile_pool(name="dsmall", bufs=4) as s_pool,
            tc.tile_pool(name="pt", bufs=2, space="PSUM") as pt_pool,
            tc.tile_pool(name="py", bufs=2, space="PSUM") as py_pool,
            tc.tile_pool(name="ps", bufs=1, space="PSUM") as ps_pool,
        ):
            w14_sb = cpool.tile([128, HID], F32R)
            w2_sb = cpool.tile([HID, 32], F32R)
            segb_sb = cpool.tile([HID, K], F32)
            mask_sb = cpool.tile([128, K], F32)
            id_sb = cpool.tile([128, 128], F32R)
            nc.sync.dma_start(w14_sb[:], w14[:])
            nc.sync.dma_start(w2_sb[:], w2x[:])
            nc.sync.dma_start(segb_sb[:], segb[:])
            nc.sync.dma_start(mask_sb[:], maskt[:])
            nc.sync.dma_start(id_sb[:], ident[:])


            scv = scr_s[:].rearrange("(k h u n) -> k h u n", k=K, h=2, u=4)
            vcv = scr_v[:].rearrange("(k h u n) -> k h u n", k=K, h=2, u=4)

            sc_state = {"buf": None, "base": -1}

            def _emit_mm2(item):
                g_sb, k, hh = item
                sg = 2 * k + hh
                g_v = g_sb[:].rearrange("p (j m) -> p j m", j=4)
                psum_s = ps_pool.tile([8, 512], F32, tag="ps")
                for u in range(4):
                    nc.tensor.matmul(
                        psum_s[:],
                        w2_sb[:, 8 * u : 8 * u + 8],
                        g_v[:, :, 128 * u : 128 * u + 128],
                        start=(u == 0), stop=(u == 3),
                    )
                if sc_state["buf"] is None:
                    sc_state["buf"] = sv_pool.tile([8, 2048], F32, tag="sc", name="sc_big")
                    sc_state["base"] = sg
                sl = sg - sc_state["base"]
                nc.vector.tensor_copy(
                    sc_state["buf"][:, 512 * sl : 512 * sl + 512], psum_s[:]
                )
                if sl == 3:
                    buf = sc_state["buf"]
                    k0 = sc_state["base"] // 2
                    # dst dims (u:partition, k:2, h:2, n:512); src free (sgl, n)
                    d_s = scv[k0 : k0 + 2].rearrange("k h u n -> u k h n")
                    d_v = vcv[k0 : k0 + 2].rearrange("k h u n -> u k h n")
                    s_s = buf[0:4, :].rearrange("u (kr h n) -> u kr h n", kr=2, h=2)
                    s_v = buf[4:8, :].rearrange("u (kr h n) -> u kr h n", kr=2, h=2)
                    nc.sync.dma_start(d_s, s_s)
                    nc.sync.dma_start(d_v, s_v)
                    sc_state["buf"] = None

            prev = []
            # ---- Phase A: per k: both halves: transpose -> mm1 -> gelu -> mm2
            for k in range(K):
                tnat = tn_pool.tile([128, 1024], F32R)  # free = (h,t,j,c)
                src = xc[:, k : k + 1, :].rearrange(
                    "(h t j p) kk c -> p h t j kk c", h=2, t=4, j=4, p=128
                ).squeeze(4)
                dst = tnat[:].rearrange("p (h t j c) -> p h t j c", h=2, t=4, j=4)
                nc.sync.dma_start(dst, src)

                for hh in range(2):
                    sg = k * 2 + hh
                    tview = tnat[:, 512 * hh : 512 * hh + 512]

                    psum_t = pt_pool.tile([128, 512], F32)
                    for t in range(4):
                        nc.tensor.transpose(
                            psum_t[:, 128 * t : 128 * t + 128].bitcast(F32R),
                            tview[:, 128 * t : 128 * t + 128],
                            id_sb[:],
                        )
                    x_t = xt_pool.tile([128, 512], F32R)
                    nc.vector.tensor_copy(x_t[:], psum_t[:].bitcast(F32R))

                    g_sb = g_pool.tile([128, 2048], F32R)
                    for half in range(2):
                        psum_y = py_pool.tile(
                            [128, 1024], F32, tag="py", name="psum_y"
                        )
                        for jj in range(2):
                            j = 2 * half + jj
                            nc.tensor.matmul(
                                psum_y[:, 512 * jj : 512 * jj + 512],
                                w14_sb[32 * j : 32 * j + 32, :],
                                x_t[32 * j : 32 * j + 32, :],
                                tile_position=(32 * j, 0),
                            )
                        nc.scalar.activation(
                            g_sb[:, 1024 * half : 1024 * half + 1024],
                            psum_y[:],
                            AF.Gelu,
                            bias=segb_sb[:, k : k + 1],
                        )

                    prev.append((g_sb, k, hh))
                    if len(prev) > 1:
                        _emit_mm2(prev.pop(0))

            while prev:
                _emit_mm2(prev.pop(0))

            # ---- Phase B: read back dense
            # dense tiles: memory layout f = 32*k + ql (k outer)
            ds = d_pool.tile([128, K * 32], F32, tag="dense")
            dv = d_pool.tile([128, K * 32], F32, tag="dense")
            nc.sync.dma_start(
                ds[:].rearrange("p (k ql) -> p k ql", k=K),
                scr_s[:].rearrange("(k p ql) -> p k ql", k=K, p=128),
            )
            nc.sync.dma_start(
                dv[:].rearrange("p (k ql) -> p k ql", k=K),
                scr_v[:].rearrange("(k p ql) -> p k ql", k=K, p=128),
            )

            p_exp = d_pool.tile([128, K * 32], F32, tag="dense")
            t_val = d_pool.tile([128, K * 32], F32, tag="dense")
            nc.scalar.activation(p_exp[:], ds[:], AF.Exp)
            nc.scalar.activation(t_val[:], dv[:], AF.Tanh, scale=0.5, bias=tanh_bias)

            w_un = d_pool.tile([128, K * 32], F32, tag="dense")
            nc.vector.tensor_tensor(
                w_un[:].rearrange("p (k ql) -> p k ql", k=K),
                p_exp[:].rearrange("p (k ql) -> p k ql", k=K),
                mask_sb[:].unsqueeze(2).broadcast_to([128, K, 32]),
                ALU.mult,
            )
            zsum = s_pool.tile([128, 32], F32, tag="small")
            nc.vector.tensor_reduce(
                zsum[:],
                w_un[:].rearrange("p (k ql) -> p ql k", k=K),
                AX.X, ALU.add,
            )
            zrec = s_pool.tile([128, 32], F32, tag="small")
            nc.vector.reciprocal(zrec[:], zsum[:])

            # wn memory layout f = 32*ql + k (matches wout row order)
            wn = d_pool.tile([128, K * 32], F32, tag="dense")
            nc.vector.tensor_tensor(
                wn[:].rearrange("p (ql k) -> p ql k", k=K),
                w_un[:].rearrange("p (k ql) -> p ql k", k=K),
                zrec[:].unsqueeze(2).broadcast_to([128, 32, K]),
                ALU.mult,
            )
            wt = d_pool.tile([128, K * 32], F32, tag="dense")
            nc.vector.tensor_tensor(
                wt[:].rearrange("p (ql k) -> p ql k", k=K),
                wn[:].rearrange("p (ql k) -> p ql k", k=K),
                t_val[:].rearrange("p (k ql) -> p ql k", k=K),
                ALU.mult,
            )
            ssum = s_pool.tile([128, 32], F32, tag="small")
            nc.vector.tensor_reduce(
                ssum[:], wt[:].rearrange("p (ql k) -> p ql k", k=K), AX.X, ALU.add
            )
            rho = s_pool.tile([128, 32], F32, tag="small")
            nc.vector.tensor_scalar(rho[:], ssum[:], 0.5, 0.5, ALU.mult, ALU.add)

            nc.sync.dma_start(
                wout[:].rearrange("(p ql) k -> p (ql k)", p=128), wn[:]
            )
            nc.sync.dma_start(rout[:].rearrange("(p ql) -> p ql", p=128), rho[:])

    nc.compile()
    return nc


_CACHE = {}
LAST_RESULT = None


def kernel(
    seg_emb, char_feat, pieces_mask, Ws1, bs1, Ws2, bs2,
    Wv1, bv1, Wv2, bv2, log_temperature, nt, nx,
):
    seg_emb = np.asarray(seg_emb, np.float32)
    char_feat = np.asarray(char_feat, np.float32)
    mask = np.asarray(pieces_mask)
    Ws1 = np.asarray(Ws1, np.float32); bs1 = np.asarray(bs1, np.float32)
    Ws2 = np.asarray(Ws2, np.float32); bs2 = np.asarray(bs2, np.float32)
    Wv1 = np.asarray(Wv1, np.float32); bv1 = np.asarray(bv1, np.float32)
    Wv2 = np.asarray(Wv2, np.float32); bv2 = np.asarray(bv2, np.float32)
    T = float(np.exp(np.float32(log_temperature)))

    maskf = mask.astype(np.float32)                      # (B, K)
    seg = seg_emb * maskf[:, :, None]                    # (B, K, H)
    W1comb = np.concatenate([Ws1[H:], Wv1[H:]], axis=1)  # (32, 128)
    w14 = np.ascontiguousarray(np.tile(W1comb, (4, 1)), dtype=np.float32)
    w2x = np.zeros((HID, 32), np.float32)
    for u in range(4):
        w2x[:H, 8 * u + u] = -Ws2[:, 0] / T
        w2x[H:, 8 * u + 4 + u] = Wv2[:, 0]
    ident = np.eye(128, dtype=np.float32)
    segb_all = []
    for b in range(B):
        sb_s = seg[b] @ Ws1[:H] + bs1                    # (K, 64)
        sb_v = seg[b] @ Wv1[:H] + bv1                    # (K, 64)
        segb_all.append(
            np.ascontiguousarray(np.concatenate([sb_s, sb_v], axis=1).T,
                                 dtype=np.float32))      # (128, K)

    tanh_bias = 0.5 * float(bv2[0])
    if tanh_bias not in _CACHE:
        _CACHE[tanh_bias] = _build_nc(tanh_bias)
    nc = _CACHE[tanh_bias]

    in_maps = []
    for c in range(NCORES):
        b = c // (NCORES // B)
        q0 = (c % (NCORES // B)) * QC
        in_maps.append({
            "xc": np.ascontiguousarray(char_feat[b, q0 : q0 + QC]),
            "w14": w14,
            "w2x": w2x,
            "segb": segb_all[b],
            "maskt": np.ascontiguousarray(
                np.tile(maskf[b][None, :], (128, 1)), dtype=np.float32),
            "ident": ident,
        })

    import os
    kw = {}
    if os.environ.get("BASS_TRACE"):
        kw = dict(trace=True, trace_cores=[0])
    global LAST_RESULT
    LAST_RESULT = run_bass_kernel_spmd(nc, in_maps, list(range(NCORES)), **kw)
    res = LAST_RESULT.results

    weights = np.empty((B, Q, K), np.float32)
    rho = np.empty((B, Q), np.float32)
    for c in range(NCORES):
        b = c // (NCORES // B)
        q0 = (c % (NCORES // B)) * QC
        weights[b, q0 : q0 + QC] = res[c]["wout"]
        rho[b, q0 : q0 + QC] = res[c]["rout"]

    output_grid = rho.reshape(B, 1, NT, NX)
    selection_weights = weights.reshape(B, NT, NX, K)
    return output_grid, selection_weights


# revision 21
# speedup vs baseline: 1.1975x; 1.0226x over previous
import sys

sys.path.insert(0, "/opt/trn_rl_repo")

import numpy as np

from concourse import bacc, bass, mybir, tile
from concourse.bass_utils import run_bass_kernel_spmd

# Problem constants (hardcoded; see spec)
B, K, H, Ch = 2, 32, 64, 32
NT, NX = 64, 256
Q = NT * NX            # 16384 queries per batch element
NCORES = 8
QC = Q * B // NCORES   # 4096 queries per core (q-slab)
HID = 2 * H            # 128: score-hidden(64) | value-hidden(64)

F32 = mybir.dt.float32
F32R = mybir.dt.float32r
AF = mybir.ActivationFunctionType
ALU = mybir.AluOpType
AX = mybir.AxisListType


def _build_nc(tanh_bias):
    nc = bacc.Bacc(None, target_bir_lowering=False)

    xc = nc.dram_tensor("xc", [QC, K, Ch], F32R, kind="ExternalInput")
    w14 = nc.dram_tensor("w14", [128, HID], F32R, kind="ExternalInput")
    w2x = nc.dram_tensor("w2x", [HID, 32], F32R, kind="ExternalInput")
    segb = nc.dram_tensor("segb", [HID, K], F32, kind="ExternalInput")
    maskt = nc.dram_tensor("maskt", [128, K], F32, kind="ExternalInput")
    ident = nc.dram_tensor("ident", [128, 128], F32R, kind="ExternalInput")

    wout = nc.dram_tensor("wout", [QC, K], F32, kind="ExternalOutput")
    rout = nc.dram_tensor("rout", [QC], F32, kind="ExternalOutput")

    scr_s = nc.dram_tensor("scr_s", [K * QC], F32, kind="Internal")
    scr_v = nc.dram_tensor("scr_v", [K * QC], F32, kind="Internal")

    with tile.TileContext(nc) as tc:
        with (
            tc.tile_pool(name="const", bufs=1) as cpool,
            tc.tile_pool(name="tn", bufs=3) as tn_pool,
            tc.tile_pool(name="xt", bufs=3) as xt_pool,
            tc.tile_pool(name="g", bufs=3) as g_pool,
            tc.tile_pool(name="sv", bufs=2) as sv_pool,
            tc.tile_pool(name="dense", bufs=4) as d_pool,
            tc.tile_pool(name="dsmall", bufs=4) as s_pool,
            tc.tile_pool(name="pt", bufs=2, space="PSUM") as pt_pool,
            tc.tile_pool(name="py", bufs=2, space="PSUM") as py_pool,
            tc.tile_pool(name="ps", bufs=2, space="PSUM") as ps_pool,
        ):
            w14_sb = cpool.tile([128, HID], F32R)
            w2_sb = cpool.tile([HID, 32], F32R)
            segb_sb = cpool.tile([HID, K], F32)
            mask_sb = cpool.tile([128, K], F32)
            id_sb = cpool.tile([128, 128], F32R)
            nc.sync.dma_start(w14_sb[:], w14[:])
            nc.sync.dma_start(w2_sb[:], w2x[:])
            nc.sync.dma_start(segb_sb[:], segb[:])
            nc.sync.dma_start(mask_sb[:], maskt[:])
            nc.sync.dma_start(id_sb[:], ident[:])


            scv = scr_s[:].rearrange("(k h u n) -> k h u n", k=K, h=2, u=4)
            vcv = scr_v[:].rearrange("(k h u n) -> k h u n", k=K, h=2, u=4)

            sc_state = {"buf": None, "base": -1}

            def _emit_mm2(item):
                g_sb, k, hh = item
                sg = 2 * k + hh
                g_v = g_sb[:].rearrange("p (j m) -> p j m", j=4)
                psum_s = ps_pool.tile([8, 512], F32, tag="ps")
                for u in range(4):
                    nc.tensor.matmul(
                        psum_s[:],
                        w2_sb[:, 8 * u : 8 * u + 8],
                        g_v[:, :, 128 * u : 128 * u + 128],
                        start=(u == 0), stop=(u == 3),
                    )
                if sc_state["buf"] is None:
                    sc_state["buf"] = sv_pool.tile([8, 2048], F32, tag="sc", name="sc_big")
                    sc_state["base"] = sg
                sl = sg - sc_state["base"]
                nc.vector.tensor_copy(
                    sc_state["buf"][:, 512 * sl : 512 * sl + 512], psum_s[:]
                )
                if sl == 3:
                    buf = sc_state["buf"]
                    k0 = sc_state["base"] // 2
                    # dst dims (u:partition, k:2, h:2, n:512); src free (sgl, n)
                    d_s = scv[k0 : k0 + 2].rearrange("k h u n -> u k h n")
                    d_v = vcv[k0 : k0 + 2].rearrange("k h u n -> u k h n")
                    s_s = buf[0:4, :].rearrange("u (kr h n) -> u kr h n", kr=2, h=2)
                    s_v = buf[4:8, :].rearrange("u (kr h n) -> u kr h n", kr=2, h=2)
                    nc.sync.dma_start(d_s, s_s)
                    nc.sync.dma_start(d_v, s_v)
                    sc_state["buf"] = None

            prev = []
            # ---- Phase A: per k: both halves: transpose -> mm1 -> gelu -> mm2
            for k in range(K):
                tnat = tn_pool.tile([128, 1024], F32R)  # free = (h,t,j,c)
                src = xc[:, k : k + 1, :].rearrange(
                    "(h t j p) kk c -> p h t j kk c", h=2, t=4, j=4, p=128
                ).squeeze(4)
                dst = tnat[:].rearrange("p (h t j c) -> p h t j c", h=2, t=4, j=4)
                nc.sync.dma_start(dst, src)

                for hh in range(2):
                    sg = k * 2 + hh
                    tview = tnat[:, 512 * hh : 512 * hh + 512]

                    psum_t = pt_pool.tile([128, 512], F32)
                    for t in range(4):
                        nc.tensor.transpose(
                            psum_t[:, 128 * t : 128 * t + 128].bitcast(F32R),
                            tview[:, 128 * t : 128 * t + 128],
                            id_sb[:],
                        )
                    x_t = xt_pool.tile([128, 512], F32R)
                    nc.vector.tensor_copy(x_t[:], psum_t[:].bitcast(F32R))

                    g_sb = g_pool.tile([128, 2048], F32R)
                    for half in range(2):
                        psum_y = py_pool.tile(
                            [128, 1024], F32, tag="py", name="psum_y"
                        )
                        for jj in range(2):
                            j = 2 * half + jj
                            nc.tensor.matmul(
                                psum_y[:, 512 * jj : 512 * jj + 512],
                                w14_sb[32 * j : 32 * j + 32, :],
                                x_t[32 * j : 32 * j + 32, :],
                                tile_position=(32 * j, 0),
                            )
                        nc.scalar.activation(
                            g_sb[:, 1024 * half : 1024 * half + 1024],
                            psum_y[:],
                            AF.Gelu,
                            bias=segb_sb[:, k : k + 1],
                        )

                    prev.append((g_sb, k, hh))
                    if len(prev) > 1:
                        _emit_mm2(prev.pop(0))

            while prev:
                _emit_mm2(prev.pop(0))

            # ---- Phase B: read back dense
            # dense tiles: memory layout f = 32*k + ql (k outer)
            ds = d_pool.tile([128, K * 32], F32, tag="dense")
            dv = d_pool.tile([128, K * 32], F32, tag="dense")
            nc.sync.dma_start(
                ds[:].rearrange("p (k ql) -> p k ql", k=K),
                scr_s[:].rearrange("(k p ql) -> p k ql", k=K, p=128),
            )
            nc.sync.dma_start(
                dv[:].rearrange("p (k ql) -> p k ql", k=K),
                scr_v[:].rearrange("(k p ql) -> p k ql", k=K, p=128),
            )

            p_exp = d_pool.tile([128, K * 32], F32, tag="dense")
            t_val = d_pool.tile([128, K * 32], F32, tag="dense")
            nc.scalar.activation(p_exp[:], ds[:], AF.Exp)
            nc.scalar.activation(t_val[:], dv[:], AF.Tanh, scale=0.5, bias=tanh_bias)

            w_un = d_pool.tile([128, K * 32], F32, tag="dense")
            nc.vector.tensor_tensor(
                w_un[:].rearrange("p (k ql) -> p k ql", k=K),
                p_exp[:].rearrange("p (k ql) -> p k ql", k=K),
                mask_sb[:].unsqueeze(2).broadcast_to([128, K, 32]),
                ALU.mult,
            )
            zsum = s_pool.tile([128, 32], F32, tag="small")
            nc.vector.tensor_reduce(
                zsum[:],
                w_un[:].rearrange("p (k ql) -> p ql k", k=K),
                AX.X, ALU.add,
            )
            zrec = s_pool.tile([128, 32], F32, tag="small")
            nc.vector.reciprocal(zrec[:], zsum[:])

            # wn memory layout f = 32*ql + k (matches wout row order)
            wn = d_pool.tile([128, K * 32], F32, tag="dense")
            nc.vector.tensor_tensor(
                wn[:].rearrange("p (ql k) -> p ql k", k=K),
                w_un[:].rearrange("p (k ql) -> p ql k", k=K),
                zrec[:].unsqueeze(2).broadcast_to([128, 32, K]),
                ALU.mult,
            )
            wt = d_pool.tile([128, K * 32], F32, tag="dense")
            nc.vector.tensor_tensor(
                wt[:].rearrange("p (ql k) -> p ql k", k=K),
                wn[:].rearrange("p (ql k) -> p ql k", k=K),
                t_val[:].rearrange("p (k ql) -> p ql k", k=K),
                ALU.mult,
            )
            ssum = s_pool.tile([128, 32], F32, tag="small")
            nc.vector.tensor_reduce(
                ssum[:], wt[:].rearrange("p (ql k) -> p ql k", k=K), AX.X, ALU.add
            )
            rho = s_pool.tile([128, 32], F32, tag="small")
            nc.vector.tensor_scalar(rho[:], ssum[:], 0.5, 0.5, ALU.mult, ALU.add)

            nc.sync.dma_start(
                wout[:].rearrange("(p ql) k -> p (ql k)", p=128), wn[:]
            )
            nc.sync.dma_start(rout[:].rearrange("(p ql) -> p ql", p=128), rho[:])

    nc.compile()
    return nc


_CACHE = {}
LAST_RESULT = None


def kernel(
    seg_emb, char_feat, pieces_mask, Ws1, bs1, Ws2, bs2,
    Wv1, bv1, Wv2, bv2, log_temperature, nt, nx,
):
    seg_emb = np.asarray(seg_emb, np.float32)
    char_feat = np.asarray(char_feat, np.float32)
    mask = np.asarray(pieces_mask)
    Ws1 = np.asarray(Ws1, np.float32); bs1 = np.asarray(bs1, np.float32)
    Ws2 = np.asarray(Ws2, np.float32); bs2 = np.asarray(bs2, np.float32)
    Wv1 = np.asarray(Wv1, np.float32); bv1 = np.asarray(bv1, np.float32)
    Wv2 = np.asarray(Wv2, np.float32); bv2 = np.asarray(bv2, np.float32)
    T = float(np.exp(np.float32(log_temperature)))

    maskf = mask.astype(np.float32)                      # (B, K)
    seg = seg_emb * maskf[:, :, None]                    # (B, K, H)
    W1comb = np.concatenate([Ws1[H:], Wv1[H:]], axis=1)  # (32, 128)
    w14 = np.ascontiguousarray(np.tile(W1comb, (4, 1)), dtype=np.float32)
    w2x = np.zeros((HID, 32), np.float32)
    for u in range(4):
        w2x[:H, 8 * u + u] = -Ws2[:, 0] / T
        w2x[H:, 8 * u + 4 + u] = Wv2[:, 0]
    ident = np.eye(128, dtype=np.float32)
    segb_all = []
    for b in range(B):
        sb_s = seg[b] @ Ws1[:H] + bs1                    # (K, 64)
        sb_v = seg[b] @ Wv1[:H] + bv1                    # (K, 64)
        segb_all.append(
            np.ascontiguousarray(np.concatenate([sb_s, sb_v], axis=1).T,
                                 dtype=np.float32))      # (128, K)

    tanh_bias = 0.5 * float(bv2[0])
    if tanh_bias not in _CACHE:
        _CACHE[tanh_bias] = _build_nc(tanh_bias)
    nc = _CACHE[tanh_bias]

    in_maps = []
    for c in range(NCORES):
        b = c // (NCORES // B)
        q0 = (c % (NCORES // B)) * QC
        in_maps.append({
            "xc": np.ascontiguousarray(char_feat[b, q0 : q0 + QC]),
            "w14": w14,
            "w2x": w2x,
            "segb": segb_all[b],
            "maskt": np.ascontiguousarray(
                np.tile(maskf[b][None, :], (128, 1)), dtype=np.float32),
            "ident": ident,
        })

    import os
    kw = {}
    if os.environ.get("BASS_TRACE"):
        kw = dict(trace=True, trace_cores=[0])
    global LAST_RESULT
    LAST_RESULT = run_bass_kernel_spmd(nc, in_maps, list(range(NCORES)), **kw)
    res = LAST_RESULT.results

    weights = np.empty((B, Q, K), np.float32)
    rho = np.empty((B, Q), np.float32)
    for c in range(NCORES):
        b = c // (NCORES // B)
        q0 = (c % (NCORES // B)) * QC
        weights[b, q0 : q0 + QC] = res[c]["wout"]
        rho[b, q0 : q0 + QC] = res[c]["rout"]

    output_grid = rho.reshape(B, 1, NT, NX)
    selection_weights = weights.reshape(B, NT, NX, K)
    return output_grid, selection_weights
